# revision 2
# baseline (speedup 1.0000x reference)
"""Trainium2 Bass kernel for nn_Attention — v2.

Reference semantics (faithful reshape WITHOUT head transpose):
  qkv = x @ w_qkv                                  # [B, N, 3*1024]
  head h <- token rows [h*128,(h+1)*128); each token n contributes 16
  sub-tokens (col-blocks cb of 64 channels); head-local attention over
  2048 sub-tokens of dim 64.

Sharding: 32 (b, head) pairs over 8 cores -> each core: 1 batch x 4 heads.
Pure data parallel, no collectives. Host preps xT (bf16) per core + full
w (bf16) — same contract as v1.

v2 layout strategy (vs v1):
- qT/kT produced DIRECTLY TRANSPOSED by the projection: lhsT =
  w[:, p*128:(p+1)*128] (a contiguous column pair (cb=2p, cb=2p+1)),
  rhs = xT -> psum [128 = dd of even cb | dd of odd cb, 512 tokens].
  Zero PE transposes.
- S matmuls contract K=64 on a 64-partition half (base 0 for even cb,
  base 64 for odd cb); cost is unchanged (cost = N columns).  lhsT and
  rhs partition bases must match, so a half-swapped duplicate qq1 of
  qq0 is made with a partition-shifted SBUF->SBUF DMA (zero PE cost).
- S psum column order is [even-cb queries | odd-cb queries] per pass;
  undone by output indexing (queries never permuted on keys' softmax).
- PV lhsT = v_ones [128 keys, 80] (64 v cols + ones col + 15 zero pad
  cols so the PV output po [80, 1024] is DMA-transposable: p%16==0).
- Tail: OT -> XBAR DMA transpose (sync queue) -> DVE recip/mul ->
  per-head [128, 1024] f32 staging -> single 512KB output DMA per head.
"""

import numpy as np
import ml_dtypes

B, N, D = 2, 2048, 1024
H_PER_CORE = 4          # head-blocks per core
ROWS = 128              # token rows per head-block
TOK = H_PER_CORE * ROWS # 512 tokens per core
DH = 64                 # head dim
CB = 16                 # col-blocks (sub-token groups) per head
PAIRS = 8               # cb pairs
KO = 8                  # k-tiles of the D=1024 contraction
SCALE = 0.125           # 64 ** -0.5
PD = 80                 # padded PV output partitions (mult of 16)
N_CORES = 8

_GRAPH = None


def build_graph():
    global _GRAPH
    if _GRAPH is not None:
        return _GRAPH

    import concourse.mybir as mybir
    import concourse.tile as tile
    from concourse import bacc
    from contextlib import ExitStack

    f32 = mybir.dt.float32
    bf16 = mybir.dt.bfloat16
    EXP = mybir.ActivationFunctionType.Exp

    nc = bacc.Bacc("TRN2", target_bir_lowering=False, debug=False,
                   num_devices=N_CORES)

    xt_dram = nc.dram_tensor("xt", [D, TOK], bf16, kind="ExternalInput")
    w_dram = nc.dram_tensor("w", [D, 3 * D], bf16, kind="ExternalInput")
    out_dram = nc.dram_tensor("out", [TOK, D], f32, kind="ExternalOutput")

    with tile.TileContext(nc) as tc, ExitStack() as ctx:
        const_pool = ctx.enter_context(tc.tile_pool(name="const", bufs=1))
        in_pool = ctx.enter_context(tc.tile_pool(name="inputs", bufs=1))
        qk_pool = ctx.enter_context(tc.tile_pool(name="qk", bufs=1))
        v_pool = ctx.enter_context(tc.tile_pool(name="v", bufs=1))
        pt_pool = ctx.enter_context(tc.tile_pool(name="pt", bufs=4))
        ot_pool = ctx.enter_context(tc.tile_pool(name="ot", bufs=3))
        ott_pool = ctx.enter_context(tc.tile_pool(name="ott", bufs=3))
        stage_pool = ctx.enter_context(tc.tile_pool(name="stage", bufs=1))
        small_pool = ctx.enter_context(tc.tile_pool(name="small", bufs=16))
        pproj = ctx.enter_context(tc.tile_pool(name="pproj", bufs=2,
                                               space="PSUM"))
        ps_pool = ctx.enter_context(tc.tile_pool(name="ps", bufs=2,
                                                 space="PSUM"))
        po_pool = ctx.enter_context(tc.tile_pool(name="po", bufs=1,
                                                 space="PSUM"))

        # warm up the exp table immediately
        warm = const_pool.tile([128, 1], f32, tag="warm")
        nc.vector.memset(warm[:], 0.0)
        nc.scalar.activation(warm[:], warm[:], EXP)

        # HAM warm-up: dummy matmuls keep the PE busy from t=0 while the
        # input DMAs stream, so the projection prefill runs at 2.4 GHz
        # instead of the cold 1.2 GHz (the HAM un-throttles after ~3.4us
        # of sustained PE activity).  Outputs are never read.
        warmW = const_pool.tile([128, 128], bf16, tag="warmW")
        warmX = const_pool.tile([128, 512], bf16, tag="warmX")
        nc.vector.memset(warmW[:], 0.0)
        nc.vector.memset(warmX[:], 0.0)

        # persistent transposed q/k: [128 = (dd|even cb, dd|odd cb), pair, tok]
        qq0 = qk_pool.tile([128, PAIRS, TOK], bf16, tag="qq0", name="qq0")
        qq1 = qk_pool.tile([128, PAIRS, TOK], bf16, tag="qq1", name="qq1")
        kk = qk_pool.tile([128, PAIRS, TOK], bf16, tag="kk", name="kk")
        # v (+ones +zero-pad) per head: [128 keys, cb, 80]
        v_ones = [v_pool.tile([128, CB, PD], bf16, tag=f"vo{t}",
                              name=f"vo{t}") for t in range(H_PER_CORE)]
        for t in range(H_PER_CORE):
            nc.vector.memset(v_ones[t][:, :, DH], 1.0)
            nc.vector.memset(v_ones[t][:, :, DH + 1:], 0.0)

        # ---- input DMA (two queues, first-consumption order) ----
        xt_sbuf = in_pool.tile([128, KO, TOK], bf16, tag="xt")
        w_sbuf = in_pool.tile([128, KO, 3 * D], bf16, tag="w")

        for _ in range(28):
            wps = pproj.tile([128, 512], f32, tag="pp")
            nc.tensor.matmul(wps[:], warmW[:], warmX[:],
                             start=True, stop=True)

        def dma_w(eng, kt, c0, c1):
            eng.dma_start(w_sbuf[:, kt, c0:c1],
                          w_dram.ap()[kt * 128:(kt + 1) * 128, c0:c1])

        for kt in range(KO):
            nc.sync.dma_start(xt_sbuf[:, kt, :],
                              xt_dram.ap()[kt * 128:(kt + 1) * 128, :])
            dma_w(nc.gpsimd, kt, 0, 512)          # q pairs 0-3
        for kt in range(KO):
            dma_w(nc.sync, kt, 1024, 1536)        # k pairs 0-3
            dma_w(nc.gpsimd, kt, 2048, 2560)      # v cols 0-7
        for kt in range(KO):
            dma_w(nc.gpsimd, kt, 2560, 3072)      # v cols 8-15
        for kt in range(KO):
            dma_w(nc.gpsimd, kt, 1536, 2048)      # k pairs 4-7
        for kt in range(KO):
            dma_w(nc.gpsimd, kt, 512, 1024)       # q pairs 4-7

        # ---- projection emitters ----
        def proj_qk(dst, wbase, p):
            ps = pproj.tile([128, 512], f32, tag="pp")
            for kt in range(KO):
                nc.tensor.matmul(ps[:],
                                 w_sbuf[:, kt, wbase + p * 128:
                                        wbase + (p + 1) * 128],
                                 xt_sbuf[:, kt, :],
                                 start=(kt == 0), stop=(kt == KO - 1))
            nc.vector.tensor_copy(dst[:, p, :], ps[:])

        def swap_pair(p):
            # partition-shifted SBUF->SBUF copies build the half-swapped
            # duplicate of qq0 (so any (cbk,cbq) parity combo can contract
            # on a matching 64-partition base).  The scalar HWDGE queue is
            # empty early, so these never sit behind input transfers.
            nc.sync.dma_start(qq1[0:64, p, :], qq0[64:128, p, :])
            nc.sync.dma_start(qq1[64:128, p, :], qq0[0:64, p, :])

        def proj_q(p):
            proj_qk(qq0, 0, p)
            swap_pair(p)

        def proj_k(p):
            proj_qk(kk, D, p)

        def proj_v(t, c):
            ps = pproj.tile([128, 512], f32, tag="pp")
            for kt in range(KO):
                nc.tensor.matmul(ps[:],
                                 xt_sbuf[:, kt, t * ROWS:(t + 1) * ROWS],
                                 w_sbuf[:, kt, 2 * D + c * 512:
                                        2 * D + (c + 1) * 512],
                                 start=(kt == 0), stop=(kt == KO - 1))
            nc.vector.tensor_copy(
                v_ones[t][:, c * 8:(c + 1) * 8, 0:DH],
                ps[:].rearrange("p (a b) -> p a b", b=DH))

        # ---- attention pass: head t, query half ih (1024 queries) ----
        def attn(t, ih, fillers=None, evac=True):
            # software-pipelined: S for cbk+1 is emitted BEFORE the exp/PV
            # of cbk, so the PE computes S(k+1) during exp(k) instead of
            # head-of-line blocking behind PV(k)'s wait on exp(k).
            fillers = fillers or {}
            po = po_pool.tile([PD, 1024], f32, tag="po")
            ps_live = {}

            def emit_S(k):
                par, p = k % 2, k // 2
                b0 = 64 * par
                qE = qq0 if par == 0 else qq1
                qO = qq1 if par == 0 else qq0
                ps = ps_pool.tile([128, 1024], f32, tag="ps")
                ps_live[k] = ps
                lhsT = kk[b0:b0 + 64, p, t * ROWS:(t + 1) * ROWS]
                nc.tensor.matmul(
                    ps[:, 0:512], lhsT,
                    qE[b0:b0 + 64, 4 * ih:4 * ih + 4,
                       t * ROWS:(t + 1) * ROWS],
                    start=True, stop=True)
                nc.tensor.matmul(
                    ps[:, 512:1024], lhsT,
                    qO[b0:b0 + 64, 4 * ih:4 * ih + 4,
                       t * ROWS:(t + 1) * ROWS],
                    start=True, stop=True)

            # 2-cbk supersteps: batch [S,S,S,S][PV,PV,PV,PV] so the PE pays
            # only 2 (not 4) full-row/64-row LDWEIGHTS drain transitions
            # per 2 cbk.  fillers run first (before this superstep's PVs
            # and the next S pair, which is legal for same-pass data).
            emit_S(0)
            emit_S(1)
            for s in range(CB // 2):
                k0, k1 = 2 * s, 2 * s + 1
                if s in fillers:
                    for f in fillers[s]:
                        f()
                if k1 + 2 < CB:
                    emit_S(k0 + 2)
                    emit_S(k1 + 2)
                for k in (k0, k1):
                    pt = pt_pool.tile([128, 1024], bf16, tag="pt")
                    nc.scalar.activation(pt[:], ps_live.pop(k)[:], EXP,
                                         scale=SCALE)
                    for sub in range(2):
                        nc.tensor.matmul(
                            po[:, sub * 512:(sub + 1) * 512],
                            v_ones[t][:, k, :],
                            pt[:, sub * 512:(sub + 1) * 512],
                            start=(k == 0), stop=(k == CB - 1))
            if not evac:
                return po
            OT = ot_pool.tile([PD, 1024], bf16, tag="OT")
            nc.vector.tensor_copy(OT[:], po[:])
            return OT

        stages = [stage_pool.tile([128, D], f32, tag=f"st{t}",
                                  name=f"st{t}") for t in range(H_PER_CORE)]

        def tail(t, ih, OT, split=False):
            # split=True: alternate transposes onto the scalar HWDGE queue
            # (only safe when ACT has drained — i.e. the final pass)
            OTt = ott_pool.tile([128, 8, PD], bf16, tag="OTt")
            for c in range(8):
                eng = nc.scalar if (split and c % 2) else nc.sync
                eng.dma_start_transpose(OTt[:, c, :],
                                        OT[:, c * 128:(c + 1) * 128])
            for c in range(8):
                cb = 2 * (4 * ih + c) if c < 4 else 2 * (4 * ih + c - 4) + 1
                recip = small_pool.tile([128, 1], f32, tag="recip")
                nc.vector.reciprocal(recip[:], OTt[:, c, DH:DH + 1])
                nc.vector.tensor_scalar_mul(
                    stages[t][:, cb * DH:(cb + 1) * DH],
                    OTt[:, c, 0:DH], recip[:])

        def tail_final(t, ih, po):
            # final pass: evacuate po chunk-wise so the transposes (split
            # across both HWDGE queues) start immediately after the last
            # PV instead of behind a monolithic 1.2us CAST.
            OT = ot_pool.tile([PD, 1024], bf16, tag="OT")
            OTt = ott_pool.tile([128, 8, PD], bf16, tag="OTt")
            for c in range(8):
                nc.vector.tensor_copy(OT[:, c * 128:(c + 1) * 128],
                                      po[:, c * 128:(c + 1) * 128])
                eng = nc.scalar if c % 2 else nc.sync
                eng.dma_start_transpose(OTt[:, c, :],
                                        OT[:, c * 128:(c + 1) * 128])
            for c in range(8):
                cb = 2 * (4 * ih + c) if c < 4 else 2 * (4 * ih + c - 4) + 1
                recip = small_pool.tile([128, 1], f32, tag="recip")
                nc.vector.reciprocal(recip[:], OTt[:, c, DH:DH + 1])
                nc.vector.tensor_scalar_mul(
                    stages[t][:, cb * DH:(cb + 1) * DH],
                    OTt[:, c, 0:DH], recip[:])

        def out(t, ih, eng):
            # stage columns [ih*512:(ih+1)*512] hold cbs 8*ih..8*ih+7
            eng.dma_start(
                out_dram.ap()[t * ROWS:(t + 1) * ROWS,
                              ih * 512:(ih + 1) * 512],
                stages[t][:, ih * 512:(ih + 1) * 512])

        # ---- program order ----
        for p in range(4):
            proj_q(p)
        proj_k(0)
        proj_v(0, 0)

        # Filler keys are SUPERSTEPS (cbk pair index 0..7).  A filler at
        # superstep s is emitted before S(2s+2)/S(2s+3) and before this
        # superstep's PVs — so proj_k(g) must sit at superstep <= g-1 and
        # v for a pass's own cbk 0 may sit at superstep 0 of that pass.
        OTs = {}
        OTs[(0, 0)] = attn(0, 0, {
            0: [lambda: proj_k(1)], 1: [lambda: proj_k(2)],
            2: [lambda: proj_k(3)], 3: [lambda: proj_k(4)],
            4: [lambda: proj_k(5), lambda: proj_v(0, 1)],
            5: [lambda: proj_k(6)], 6: [lambda: proj_k(7)]})
        tail(0, 0, OTs[(0, 0)])
        out(0, 0, nc.gpsimd)
        OTs[(1, 0)] = attn(1, 0, {
            0: [lambda: proj_v(1, 0)], 2: [lambda: proj_v(1, 1)],
            4: [lambda: proj_q(4)], 6: [lambda: proj_q(5)]})
        tail(1, 0, OTs[(1, 0)])
        out(1, 0, nc.gpsimd)
        OTs[(2, 0)] = attn(2, 0, {
            0: [lambda: proj_v(2, 0)], 2: [lambda: proj_v(2, 1)],
            4: [lambda: proj_q(6)], 6: [lambda: proj_q(7)]})
        tail(2, 0, OTs[(2, 0)])
        out(2, 0, nc.gpsimd)
        OTs[(3, 0)] = attn(3, 0, {
            0: [lambda: proj_v(3, 0)], 2: [lambda: proj_v(3, 1)]})
        tail(3, 0, OTs[(3, 0)])
        out(3, 0, nc.gpsimd)
        OTs[(0, 1)] = attn(0, 1)
        tail(0, 1, OTs[(0, 1)])
        out(0, 1, nc.gpsimd)
        OTs[(1, 1)] = attn(1, 1)
        tail(1, 1, OTs[(1, 1)])
        out(1, 1, nc.gpsimd)
        OTs[(2, 1)] = attn(2, 1)
        tail(2, 1, OTs[(2, 1)])
        out(2, 1, nc.gpsimd)
        po_fin = attn(3, 1, evac=False)
        tail_final(3, 1, po_fin)
        out(3, 1, nc.sync)

    nc.compile()
    _GRAPH = nc
    return nc


def make_in_maps(x, w_qkv):
    w_bf = np.ascontiguousarray(w_qkv).astype(ml_dtypes.bfloat16)
    maps = []
    for c in range(N_CORES):
        b = c // 4
        r0 = (c % 4) * TOK
        xt = np.ascontiguousarray(
            x[b, r0:r0 + TOK, :].T).astype(ml_dtypes.bfloat16)
        maps.append({"xt": xt, "w": w_bf})
    return maps


def assemble_out(results):
    out = np.empty((B, N, D), dtype=np.float32)
    for c in range(N_CORES):
        b = c // 4
        r0 = (c % 4) * TOK
        out[b, r0:r0 + TOK, :] = results[c]["out"]
    return out


def kernel(x, w_qkv):
    from concourse import bass_utils
    nc = build_graph()
    res = bass_utils.run_bass_kernel_spmd(
        nc, make_in_maps(np.asarray(x), np.asarray(w_qkv)),
        list(range(N_CORES)))
    return assemble_out(res.results)


# revision 3
# speedup vs baseline: 1.1606x; 1.1606x over previous
"""Trainium2 Bass kernel for nn_Attention — v2.

Reference semantics (faithful reshape WITHOUT head transpose):
  qkv = x @ w_qkv                                  # [B, N, 3*1024]
  head h <- token rows [h*128,(h+1)*128); each token n contributes 16
  sub-tokens (col-blocks cb of 64 channels); head-local attention over
  2048 sub-tokens of dim 64.

Sharding: 32 (b, head) pairs over 8 cores -> each core: 1 batch x 4 heads.
Pure data parallel, no collectives. Host preps xT (bf16) per core + full
w (bf16) — same contract as v1.

v2 layout strategy (vs v1):
- qT/kT produced DIRECTLY TRANSPOSED by the projection: lhsT =
  w[:, p*128:(p+1)*128] (a contiguous column pair (cb=2p, cb=2p+1)),
  rhs = xT -> psum [128 = dd of even cb | dd of odd cb, 512 tokens].
  Zero PE transposes.
- S matmuls contract K=64 on a 64-partition half (base 0 for even cb,
  base 64 for odd cb); cost is unchanged (cost = N columns).  lhsT and
  rhs partition bases must match, so a half-swapped duplicate qq1 of
  qq0 is made with a partition-shifted SBUF->SBUF DMA (zero PE cost).
- S psum column order is [even-cb queries | odd-cb queries] per pass;
  undone by output indexing (queries never permuted on keys' softmax).
- PV lhsT = v_ones [128 keys, 80] (64 v cols + ones col + 15 zero pad
  cols so the PV output po [80, 1024] is DMA-transposable: p%16==0).
- Tail: OT -> XBAR DMA transpose (sync queue) -> DVE recip/mul ->
  per-head [128, 1024] f32 staging -> single 512KB output DMA per head.
"""

import numpy as np
import ml_dtypes

B, N, D = 2, 2048, 1024
H_PER_CORE = 4          # head-blocks per core
ROWS = 128              # token rows per head-block
TOK = H_PER_CORE * ROWS # 512 tokens per core
DH = 64                 # head dim
CB = 16                 # col-blocks (sub-token groups) per head
PAIRS = 8               # cb pairs
KO = 8                  # k-tiles of the D=1024 contraction
SCALE = 0.125           # 64 ** -0.5
PD = 80                 # padded PV output partitions (mult of 16)
N_CORES = 8

_GRAPH = None


def build_graph():
    global _GRAPH
    if _GRAPH is not None:
        return _GRAPH

    import concourse.mybir as mybir
    import concourse.tile as tile
    from concourse import bacc
    from contextlib import ExitStack

    f32 = mybir.dt.float32
    bf16 = mybir.dt.bfloat16
    EXP = mybir.ActivationFunctionType.Exp

    nc = bacc.Bacc("TRN2", target_bir_lowering=False, debug=False,
                   num_devices=N_CORES)

    xt_dram = nc.dram_tensor("xt", [D, TOK], bf16, kind="ExternalInput")
    w_dram = nc.dram_tensor("w", [D, 3 * D], bf16, kind="ExternalInput")
    out_dram = nc.dram_tensor("out", [TOK, D], f32, kind="ExternalOutput")

    with tile.TileContext(nc) as tc, ExitStack() as ctx:
        const_pool = ctx.enter_context(tc.tile_pool(name="const", bufs=1))
        in_pool = ctx.enter_context(tc.tile_pool(name="inputs", bufs=1))
        qk_pool = ctx.enter_context(tc.tile_pool(name="qk", bufs=1))
        v_pool = ctx.enter_context(tc.tile_pool(name="v", bufs=1))
        pt_pool = ctx.enter_context(tc.tile_pool(name="pt", bufs=4))
        ot_pool = ctx.enter_context(tc.tile_pool(name="ot", bufs=3))
        ott_pool = ctx.enter_context(tc.tile_pool(name="ott", bufs=3))
        stage_pool = ctx.enter_context(tc.tile_pool(name="stage", bufs=1))
        small_pool = ctx.enter_context(tc.tile_pool(name="small", bufs=16))
        pproj = ctx.enter_context(tc.tile_pool(name="pproj", bufs=2,
                                               space="PSUM"))
        ps_pool = ctx.enter_context(tc.tile_pool(name="ps", bufs=2,
                                                 space="PSUM"))
        po_pool = ctx.enter_context(tc.tile_pool(name="po", bufs=1,
                                                 space="PSUM"))

        # warm up the exp table immediately
        warm = const_pool.tile([128, 1], f32, tag="warm")
        nc.vector.memset(warm[:], 0.0)
        nc.scalar.activation(warm[:], warm[:], EXP)

        # No PE warm-up matmuls: the first projection pair is DMA-chunk
        # paced (~650ns/MM > cold MM cost), so the HAM warms for free
        # during it; dummy warm-ups in front only delay the prefill.

        # persistent transposed q/k: [128 = (dd|even cb, dd|odd cb), pair, tok]
        qq0 = qk_pool.tile([128, PAIRS, TOK], bf16, tag="qq0", name="qq0")
        qq1 = qk_pool.tile([128, PAIRS, TOK], bf16, tag="qq1", name="qq1")
        kk = qk_pool.tile([128, PAIRS, TOK], bf16, tag="kk", name="kk")
        # v (+ones +zero-pad) per head: [128 keys, cb, 80]
        v_ones = [v_pool.tile([128, CB, PD], bf16, tag=f"vo{t}",
                              name=f"vo{t}") for t in range(H_PER_CORE)]
        for t in range(H_PER_CORE):
            nc.vector.memset(v_ones[t][:, :, DH], 1.0)
            nc.vector.memset(v_ones[t][:, :, DH + 1:], 0.0)

        # ---- input DMA (two queues, first-consumption order) ----
        xt_sbuf = in_pool.tile([128, KO, TOK], bf16, tag="xt")
        w_sbuf = in_pool.tile([128, KO, 3 * D], bf16, tag="w")

        def dma_w(eng, kt, c0, c1):
            eng.dma_start(w_sbuf[:, kt, c0:c1],
                          w_dram.ap()[kt * 128:(kt + 1) * 128, c0:c1])

        for kt in range(KO):
            nc.sync.dma_start(xt_sbuf[:, kt, :],
                              xt_dram.ap()[kt * 128:(kt + 1) * 128, :])
            dma_w(nc.gpsimd, kt, 0, 512)          # q pairs 0-3
        for kt in range(KO):
            dma_w(nc.sync, kt, 1024, 1536)        # k pairs 0-3
            dma_w(nc.gpsimd, kt, 2048, 2560)      # v cols 0-7
        for kt in range(KO):
            dma_w(nc.gpsimd, kt, 2560, 3072)      # v cols 8-15
        for kt in range(KO):
            dma_w(nc.gpsimd, kt, 1536, 2048)      # k pairs 4-7
        for kt in range(KO):
            dma_w(nc.gpsimd, kt, 512, 1024)       # q pairs 4-7

        # ---- projection emitters ----
        def proj_qk(dst, wbase, p):
            ps = pproj.tile([128, 512], f32, tag="pp")
            for kt in range(KO):
                nc.tensor.matmul(ps[:],
                                 w_sbuf[:, kt, wbase + p * 128:
                                        wbase + (p + 1) * 128],
                                 xt_sbuf[:, kt, :],
                                 start=(kt == 0), stop=(kt == KO - 1))
            nc.vector.tensor_copy(dst[:, p, :], ps[:])

        def swap_pair(p):
            # partition-shifted SBUF->SBUF copies build the half-swapped
            # duplicate of qq0 (so any (cbk,cbq) parity combo can contract
            # on a matching 64-partition base).  The scalar HWDGE queue is
            # empty early, so these never sit behind input transfers.
            nc.sync.dma_start(qq1[0:64, p, :], qq0[64:128, p, :])
            nc.sync.dma_start(qq1[64:128, p, :], qq0[0:64, p, :])

        def proj_q(p):
            proj_qk(qq0, 0, p)
            swap_pair(p)

        def proj_k(p):
            proj_qk(kk, D, p)

        def proj_v(t, c):
            ps = pproj.tile([128, 512], f32, tag="pp")
            for kt in range(KO):
                nc.tensor.matmul(ps[:],
                                 xt_sbuf[:, kt, t * ROWS:(t + 1) * ROWS],
                                 w_sbuf[:, kt, 2 * D + c * 512:
                                        2 * D + (c + 1) * 512],
                                 start=(kt == 0), stop=(kt == KO - 1))
            nc.vector.tensor_copy(
                v_ones[t][:, c * 8:(c + 1) * 8, 0:DH],
                ps[:].rearrange("p (a b) -> p a b", b=DH))

        # ---- attention pass: head t, query half ih (1024 queries) ----
        def attn(t, ih, fillers=None, evac=True):
            # software-pipelined: S for cbk+1 is emitted BEFORE the exp/PV
            # of cbk, so the PE computes S(k+1) during exp(k) instead of
            # head-of-line blocking behind PV(k)'s wait on exp(k).
            fillers = fillers or {}
            po = po_pool.tile([PD, 1024], f32, tag="po")
            ps_live = {}

            def emit_S(k):
                par, p = k % 2, k // 2
                b0 = 64 * par
                qE = qq0 if par == 0 else qq1
                qO = qq1 if par == 0 else qq0
                ps = ps_pool.tile([128, 1024], f32, tag="ps")
                ps_live[k] = ps
                lhsT = kk[b0:b0 + 64, p, t * ROWS:(t + 1) * ROWS]
                nc.tensor.matmul(
                    ps[:, 0:512], lhsT,
                    qE[b0:b0 + 64, 4 * ih:4 * ih + 4,
                       t * ROWS:(t + 1) * ROWS],
                    start=True, stop=True)
                nc.tensor.matmul(
                    ps[:, 512:1024], lhsT,
                    qO[b0:b0 + 64, 4 * ih:4 * ih + 4,
                       t * ROWS:(t + 1) * ROWS],
                    start=True, stop=True)

            # 2-cbk supersteps: batch [S,S,S,S][PV,PV,PV,PV] so the PE pays
            # only 2 (not 4) full-row/64-row LDWEIGHTS drain transitions
            # per 2 cbk.  fillers run first (before this superstep's PVs
            # and the next S pair, which is legal for same-pass data).
            emit_S(0)
            emit_S(1)
            for s in range(CB // 2):
                k0, k1 = 2 * s, 2 * s + 1
                if s in fillers:
                    for f in fillers[s]:
                        f()
                if k1 + 2 < CB:
                    emit_S(k0 + 2)
                    emit_S(k1 + 2)
                for k in (k0, k1):
                    pt = pt_pool.tile([128, 1024], bf16, tag="pt")
                    nc.scalar.activation(pt[:], ps_live.pop(k)[:], EXP,
                                         scale=SCALE)
                    for sub in range(2):
                        nc.tensor.matmul(
                            po[:, sub * 512:(sub + 1) * 512],
                            v_ones[t][:, k, :],
                            pt[:, sub * 512:(sub + 1) * 512],
                            start=(k == 0), stop=(k == CB - 1))
            if not evac:
                return po
            OT = ot_pool.tile([PD, 1024], bf16, tag="OT")
            nc.vector.tensor_copy(OT[:], po[:])
            return OT

        stages = [stage_pool.tile([128, D], f32, tag=f"st{t}",
                                  name=f"st{t}") for t in range(H_PER_CORE)]

        def tail(t, ih, OT, split=False):
            # split=True: alternate transposes onto the scalar HWDGE queue
            # (only safe when ACT has drained — i.e. the final pass)
            OTt = ott_pool.tile([128, 8, PD], bf16, tag="OTt")
            for c in range(8):
                eng = nc.scalar if (split and c % 2) else nc.sync
                eng.dma_start_transpose(OTt[:, c, :],
                                        OT[:, c * 128:(c + 1) * 128])
            for c in range(8):
                cb = 2 * (4 * ih + c) if c < 4 else 2 * (4 * ih + c - 4) + 1
                recip = small_pool.tile([128, 1], f32, tag="recip")
                nc.vector.reciprocal(recip[:], OTt[:, c, DH:DH + 1])
                nc.vector.tensor_scalar_mul(
                    stages[t][:, cb * DH:(cb + 1) * DH],
                    OTt[:, c, 0:DH], recip[:])

        def tail_final(t, ih, po):
            # final pass: evacuate po chunk-wise so the transposes (split
            # across both HWDGE queues) start immediately after the last
            # PV instead of behind a monolithic 1.2us CAST.
            OT = ot_pool.tile([PD, 1024], bf16, tag="OT")
            OTt = ott_pool.tile([128, 8, PD], bf16, tag="OTt")
            for c in range(8):
                nc.vector.tensor_copy(OT[:, c * 128:(c + 1) * 128],
                                      po[:, c * 128:(c + 1) * 128])
                eng = nc.scalar if c % 2 else nc.sync
                eng.dma_start_transpose(OTt[:, c, :],
                                        OT[:, c * 128:(c + 1) * 128])
            for c in range(8):
                cb = 2 * (4 * ih + c) if c < 4 else 2 * (4 * ih + c - 4) + 1
                recip = small_pool.tile([128, 1], f32, tag="recip")
                nc.vector.reciprocal(recip[:], OTt[:, c, DH:DH + 1])
                nc.vector.tensor_scalar_mul(
                    stages[t][:, cb * DH:(cb + 1) * DH],
                    OTt[:, c, 0:DH], recip[:])

        def out(t, ih, eng):
            # stage columns [ih*512:(ih+1)*512] hold cbs 8*ih..8*ih+7
            eng.dma_start(
                out_dram.ap()[t * ROWS:(t + 1) * ROWS,
                              ih * 512:(ih + 1) * 512],
                stages[t][:, ih * 512:(ih + 1) * 512])

        # ---- program order ----
        for p in range(4):
            proj_q(p)
        proj_k(0)
        proj_v(0, 0)

        # Filler keys are SUPERSTEPS (cbk pair index 0..7).  A filler at
        # superstep s is emitted before S(2s+2)/S(2s+3) and before this
        # superstep's PVs — so proj_k(g) must sit at superstep <= g-1 and
        # v for a pass's own cbk 0 may sit at superstep 0 of that pass.
        OTs = {}
        OTs[(0, 0)] = attn(0, 0, {
            0: [lambda: proj_k(1)], 1: [lambda: proj_k(2)],
            2: [lambda: proj_k(3)], 3: [lambda: proj_k(4)],
            4: [lambda: proj_k(5), lambda: proj_v(0, 1)],
            5: [lambda: proj_k(6)], 6: [lambda: proj_k(7)]})
        tail(0, 0, OTs[(0, 0)])
        out(0, 0, nc.gpsimd)
        OTs[(1, 0)] = attn(1, 0, {
            0: [lambda: proj_v(1, 0)], 2: [lambda: proj_v(1, 1)],
            4: [lambda: proj_q(4)], 6: [lambda: proj_q(5)]})
        tail(1, 0, OTs[(1, 0)])
        out(1, 0, nc.gpsimd)
        OTs[(2, 0)] = attn(2, 0, {
            0: [lambda: proj_v(2, 0)], 2: [lambda: proj_v(2, 1)],
            4: [lambda: proj_q(6)], 6: [lambda: proj_q(7)]})
        tail(2, 0, OTs[(2, 0)])
        out(2, 0, nc.gpsimd)
        OTs[(3, 0)] = attn(3, 0, {
            0: [lambda: proj_v(3, 0)], 2: [lambda: proj_v(3, 1)]})
        tail(3, 0, OTs[(3, 0)])
        out(3, 0, nc.gpsimd)
        OTs[(0, 1)] = attn(0, 1)
        tail(0, 1, OTs[(0, 1)])
        out(0, 1, nc.gpsimd)
        OTs[(1, 1)] = attn(1, 1)
        tail(1, 1, OTs[(1, 1)])
        out(1, 1, nc.gpsimd)
        OTs[(2, 1)] = attn(2, 1)
        tail(2, 1, OTs[(2, 1)])
        out(2, 1, nc.gpsimd)
        po_fin = attn(3, 1, evac=False)
        tail_final(3, 1, po_fin)
        out(3, 1, nc.sync)

    nc.compile()
    _GRAPH = nc
    return nc


def make_in_maps(x, w_qkv):
    w_bf = np.ascontiguousarray(w_qkv).astype(ml_dtypes.bfloat16)
    maps = []
    for c in range(N_CORES):
        b = c // 4
        r0 = (c % 4) * TOK
        xt = np.ascontiguousarray(
            x[b, r0:r0 + TOK, :].T).astype(ml_dtypes.bfloat16)
        maps.append({"xt": xt, "w": w_bf})
    return maps


def assemble_out(results):
    out = np.empty((B, N, D), dtype=np.float32)
    for c in range(N_CORES):
        b = c // 4
        r0 = (c % 4) * TOK
        out[b, r0:r0 + TOK, :] = results[c]["out"]
    return out


def kernel(x, w_qkv):
    from concourse import bass_utils
    nc = build_graph()
    res = bass_utils.run_bass_kernel_spmd(
        nc, make_in_maps(np.asarray(x), np.asarray(w_qkv)),
        list(range(N_CORES)))
    return assemble_out(res.results)
